# revision 16
# baseline (speedup 1.0000x reference)
"""Trainium2 Bass kernel for nn_CorrAttentionBias.

Computes out = where(row/col masked, NEG, attn + neigh_band_bias + sink_bias)
for attn_scores [2, 16, 2048, 2048] f32, sharded over (batch, head) across
8 NeuronCores (4 heads of one batch per core).

Mask-aware packing: masked rows (~50%) have constant-NEG output and never
need their attn values, so the host packs only the unmasked rows (gathered,
head-transposed to [NU, H_PER, L] so each packed row is 32 KiB contiguous)
and the device:
  - reads the packed rows only          (~35 MB instead of 67 MB),
  - computes bias + col-mask on them    (halves the vector-engine work),
  - stores them contiguously to out[0:NU_max],
  - streams a constant NEG block to out[NU_max:] for the masked rows
    (pure DMA from SBUF, no compute dependencies).
The host unshard scatters rows back through the permutation; every output
byte is produced on device.

All DMA transfers use exactly 128 partitions: partial-partition descriptors
execute on a single DMA engine (~27 GB/s) instead of spreading across all
16, so the last block is anchored at nu_max-128 and overlapping rows are
computed/stored twice with identical bytes (same for the NEG region).

The program is compiled inside kernel() after the mask is known; it depends
on the mask only through block offsets and band column windows. SPMD across
cores requires one shape for both batches: block structure uses NU_max and
the smaller batch's packed buffer is padded (host ignores junk rows).

Per packed row-block of 128 rows (orig rows i = pi[k0 + p], sorted):
  bias[p, j] = round(BETA * round(cs[i]*cs[j]))            (ACT, 2 ops)
  bias[p, i-1] += sub[i]; bias[p, i+1] += sup[i]           (iota==t compare,
                               window-limited since pi is sorted)
  out[p, j] = min(attn[p, j] + bias[p, j], maskval[j])     (exact NEG via min)
All rounding matches the jax reference bitwise (NEG = -1e5 >> |attn+bias|).
"""

import sys

sys.path.insert(0, "/opt/trn_rl_repo")

from contextlib import ExitStack

import numpy as np

import concourse.bass as bass
import concourse.tile as tile
from concourse import bacc, mybir
from concourse.bass_utils import run_bass_kernel_spmd

ALPHA = np.float32(0.5)
BETA = np.float32(0.1)
NEG = np.float32(-100000.0)
BIG = np.float32(3.0e38)

B, H, L = 2, 16, 2048
N_CORES = 8
H_PER = (B * H) // N_CORES  # 4 heads per core
P = 128  # partitions per row-block
K = 5  # per-row values: cs, t1, v1, t2, v2

FP = mybir.dt.float32


def _block_starts(n, base=0):
    """128-row block start offsets covering [base, base+n), last block
    anchored at base+n-128 (overlap re-writes identical bytes)."""
    if n <= 0:
        return []
    if n <= P:
        return [base]
    starts = list(range(base, base + n - P, P))
    starts.append(base + n - P)
    return sorted(set(starts))


def _build_program(nu_max, n_rows, starts, m_starts, windows, trace_sim=False):
    nb = len(starts)
    wmax = max((wn for _, wn in windows), default=1)

    nc = bacc.Bacc(
        "TRN2",
        target_bir_lowering=False,
        debug=False,
        num_devices=N_CORES,
    )

    nu_buf = max(nu_max, P)  # host pads packed rows to >=128
    attn_d = nc.dram_tensor("attn", [nu_buf, H_PER, L], FP, kind="ExternalInput").ap()
    vecs_d = nc.dram_tensor("vecs", [P, nb * K], FP, kind="ExternalInput").ap()
    # rowconsts[0] = c_sink, [1] = maskval, [2] = iota
    rowconsts_d = nc.dram_tensor("rowconsts", [3, L], FP, kind="ExternalInput").ap()
    out_d = nc.dram_tensor("out", [n_rows, H_PER, L], FP, kind="ExternalOutput").ap()

    attn_rr = attn_d.rearrange("r h c -> r (h c)")
    out_rr = out_d.rearrange("r h c -> r (h c)")

    with tile.TileContext(nc, trace_sim=trace_sim) as tc, ExitStack() as ctx:
        const_pool = ctx.enter_context(tc.tile_pool(name="const", bufs=1))
        prep_pool = ctx.enter_context(tc.tile_pool(name="prep", bufs=2))
        band_pool = ctx.enter_context(tc.tile_pool(name="band", bufs=1))
        a_pool = ctx.enter_context(tc.tile_pool(name="a", bufs=3))

        vecs_sb = const_pool.tile([P, nb * K], FP, tag="vecs")
        nc.sync.dma_start(out=vecs_sb[:, :], in_=vecs_d[:, :])
        # one staging row reused for the three broadcast constants
        bcs = []
        for i in range(3):
            crow = band_pool.tile([1, L], FP, tag="crow")
            nc.sync.dma_start(out=crow[:, :], in_=rowconsts_d[i : i + 1, :])
            bc = const_pool.tile([P, L], FP, tag=f"bc{i}")
            nc.gpsimd.partition_broadcast(bc[:, :], crow[0:1, :])
            bcs.append(bc)
        csink_bc, maskval_bc, iota_bc = bcs

        # constant NEG tile for masked-row output blocks; full [128, 4L]
        # width so NEG store packets are 32 KiB like every other stream
        # (smaller packets get a smaller share of DMA-engine arbitration).
        # Issued on the scalar queue, interleaved with the bias ACT ops.
        neg_stores = []
        if m_starts:
            neg_t = const_pool.tile([P, H_PER * L], FP, tag="neg")
            nc.gpsimd.memset(neg_t[:, :], float(NEG))
            neg_stores = [
                (out_rr[r0 : r0 + P, :], neg_t[:, :]) for r0 in m_starts
            ]
        # spread NEG stores across the block loop (scalar engine issues them)
        neg_per_block = [[] for _ in range(max(nb, 1))]
        for i, ns in enumerate(neg_stores):
            neg_per_block[(i * max(nb, 1)) // max(len(neg_stores), 1)].append(ns)
        if nb == 0:
            for dst, src in neg_stores:
                nc.scalar.dma_start(out=dst, in_=src)

        for r, k0 in enumerate(starts):
            cs_col = vecs_sb[:, K * r + 0 : K * r + 1]
            t1_col = vecs_sb[:, K * r + 1 : K * r + 2]
            v1_col = vecs_sb[:, K * r + 2 : K * r + 3]
            t2_col = vecs_sb[:, K * r + 3 : K * r + 4]
            v2_col = vecs_sb[:, K * r + 4 : K * r + 5]
            ws, wn = windows[r]

            # sink bias, bitwise-matching reference: round(si*sj) then *BETA
            bias_t = prep_pool.tile([P, L], FP, tag="bias")
            nc.scalar.activation(
                out=bias_t[:, :],
                in_=csink_bc[:, :],
                func=mybir.ActivationFunctionType.Copy,
                scale=cs_col,
            )
            nc.scalar.activation(
                out=bias_t[:, :],
                in_=bias_t[:, :],
                func=mybir.ActivationFunctionType.Copy,
                scale=float(BETA),
            )
            # neighbor band: row i contributes sub[i]@col i-1, sup[i]@col i+1.
            # Packed rows are sorted, so cols live in a narrow window.
            if wn > 0:
                band1 = band_pool.tile([P, wmax], FP, tag="band1")
                nc.vector.tensor_scalar(
                    out=band1[:, :wn],
                    in0=iota_bc[:, ws : ws + wn],
                    scalar1=t1_col,
                    scalar2=v1_col,
                    op0=mybir.AluOpType.is_equal,
                    op1=mybir.AluOpType.mult,
                )
                band2 = band_pool.tile([P, wmax], FP, tag="band2")
                nc.vector.tensor_scalar(
                    out=band2[:, :wn],
                    in0=iota_bc[:, ws : ws + wn],
                    scalar1=t2_col,
                    scalar2=v2_col,
                    op0=mybir.AluOpType.is_equal,
                    op1=mybir.AluOpType.mult,
                )
                bias_win = bias_t[:, ws : ws + wn]
                nc.vector.tensor_tensor(
                    out=bias_win, in0=bias_win, in1=band1[:, :wn],
                    op=mybir.AluOpType.add,
                )
                nc.vector.tensor_tensor(
                    out=bias_win, in0=bias_win, in1=band2[:, :wn],
                    op=mybir.AluOpType.add,
                )

            # NEG stores ride the scalar queue, spread across blocks
            for dst, src in neg_per_block[r]:
                nc.scalar.dma_start(out=dst, in_=src)

            a_t = a_pool.tile([P, H_PER * L], FP, tag="a")
            nc.sync.dma_start(out=a_t[:, :], in_=attn_rr[k0 : k0 + P, :])
            # bias adds for h2/h3 go to gpsimd (its tensor_tensor only
            # supports add/mult); DVE does the other adds plus all mins.
            # gpsimd's adds overlap DVE's h0/h1 work.
            for h in (2, 3):
                a_h = a_t[:, h * L : (h + 1) * L]
                nc.gpsimd.tensor_tensor(
                    out=a_h, in0=a_h, in1=bias_t[:, :], op=mybir.AluOpType.add
                )
            for h in (0, 1):
                a_h = a_t[:, h * L : (h + 1) * L]
                nc.vector.tensor_tensor(
                    out=a_h, in0=a_h, in1=bias_t[:, :], op=mybir.AluOpType.add
                )
                nc.vector.tensor_tensor(
                    out=a_h, in0=a_h, in1=maskval_bc[:, :], op=mybir.AluOpType.min
                )
            for h in (2, 3):
                a_h = a_t[:, h * L : (h + 1) * L]
                nc.vector.tensor_tensor(
                    out=a_h, in0=a_h, in1=maskval_bc[:, :], op=mybir.AluOpType.min
                )
            nc.gpsimd.dma_start(out=out_rr[k0 : k0 + P, :], in_=a_t[:, :])

    nc.compile()
    return nc


def _band_vecs(c_local_b):
    """Per-row band values, bitwise-matching the reference's overlapping
    slice assignments."""
    sub = np.zeros(L, np.float32)
    sub[1] = c_local_b[1]
    sub[L - 1] = c_local_b[L - 1]
    sub[2 : L - 1] = c_local_b[1 : L - 2]
    sup = np.zeros(L, np.float32)
    sup[: L - 1] = c_local_b[1:]
    return ALPHA * sub, ALPHA * sup


def _host_prep(attn_scores, c_local, c_sink, mask):
    attn_scores = np.asarray(attn_scores, dtype=np.float32)
    c_local = np.asarray(c_local, dtype=np.float32)
    c_sink = np.asarray(c_sink, dtype=np.float32)
    mask = np.asarray(mask, dtype=bool)

    unm = [np.flatnonzero(~mask[b]) for b in range(B)]
    msk = [np.flatnonzero(mask[b]) for b in range(B)]
    nu = [len(u) for u in unm]
    nu_max = max(nu)
    nm_max = max(L - n for n in nu)
    nu_buf = max(nu_max, P)
    starts = _block_starts(nu_max)
    neg_base = nu_max if nu_max > P else (P if nu_max else 0)
    m_starts = _block_starts(max(nm_max, P if nm_max else 0), base=neg_base)
    n_rows = (m_starts[-1] + P) if m_starts else (starts[-1] + P if starts else 0)
    nb = len(starts)

    # padded permutations (junk rows read real data; host ignores them)
    pis = []
    for b in range(B):
        pi = np.empty(nu_buf, np.int64)
        pi[: nu[b]] = unm[b]
        if nu[b] < nu_buf:
            pi[nu[b] :] = unm[b][-1] if nu[b] else 0
        pis.append(pi)

    # band windows per block: union over batches so one program fits both
    windows = []
    for k0 in starts:
        ws, we = L, -1
        for b in range(B):
            blk = pis[b][k0 : k0 + P]
            ws = min(ws, max(int(blk.min()) - 1, 0))
            we = max(we, min(int(blk.max()) + 1, L - 1))
        windows.append((ws, we - ws + 1))

    in_maps = []
    for c in range(N_CORES):
        b = c // (N_CORES // B)
        h0 = H_PER * (c % (N_CORES // B))
        pi = pis[b]
        sub, sup = _band_vecs(c_local[b])

        vecs = np.zeros((P, nb * K), np.float32)
        for r, k0 in enumerate(starts):
            rows = pi[k0 : k0 + P]
            vecs[:, K * r + 0] = c_sink[b][rows]
            vecs[:, K * r + 1] = (rows - 1).astype(np.float32)
            vecs[:, K * r + 2] = sub[rows]
            vecs[:, K * r + 3] = (rows + 1).astype(np.float32)
            vecs[:, K * r + 4] = sup[rows]

        maskval = np.where(mask[b], NEG, BIG).astype(np.float32)
        rowconsts = np.stack(
            [c_sink[b], maskval, np.arange(L, dtype=np.float32)], axis=0
        )
        # packed rows, head-transposed: [nu_buf, H_PER, L]
        attn_packed = np.ascontiguousarray(
            attn_scores[b, h0 : h0 + H_PER][:, pi, :].transpose(1, 0, 2)
        )
        in_maps.append(
            {
                "attn": attn_packed,
                "vecs": np.ascontiguousarray(vecs),
                "rowconsts": np.ascontiguousarray(rowconsts),
            }
        )
    shape_key = (nu_max, n_rows, tuple(starts), tuple(m_starts), tuple(windows))
    return in_maps, shape_key, (unm, msk, nu, neg_base)


_PROGRAM_CACHE = {}


def kernel(attn_scores, c_local, c_sink, mask, _trace=False, _trace_kwargs=None):
    in_maps, shape_key, (unm, msk, nu, neg_base) = _host_prep(
        attn_scores, c_local, c_sink, mask
    )
    nu_max, n_rows, starts, m_starts, windows = shape_key
    if shape_key not in _PROGRAM_CACHE:
        _PROGRAM_CACHE.clear()
        _PROGRAM_CACHE[shape_key] = _build_program(
            nu_max, n_rows, list(starts), list(m_starts), list(windows)
        )
    nc = _PROGRAM_CACHE[shape_key]

    res = run_bass_kernel_spmd(
        nc,
        in_maps,
        list(range(N_CORES)),
        trace=_trace,
        **(_trace_kwargs or {}),
    )
    out = np.empty((B, H, L, L), dtype=np.float32)
    for c in range(N_CORES):
        b = c // (N_CORES // B)
        h0 = H_PER * (c % (N_CORES // B))
        dev = res.results[c]["out"]  # [n_rows, H_PER, L]
        out[b, h0 : h0 + H_PER, unm[b], :] = dev[: nu[b]]
        out[b, h0 : h0 + H_PER, msk[b], :] = dev[neg_base : neg_base + (L - nu[b])]
    kernel.last_results = res
    return out


# revision 17
# speedup vs baseline: 1.4247x; 1.4247x over previous
"""Trainium2 Bass kernel for nn_CorrAttentionBias — v5: row AND column packing.

out = where(row/col masked, NEG, attn + band_bias + sink_bias), sharded over
(batch, head) across 8 cores (4 heads of one batch each).

Both the row mask and the column mask use the same [B, L] mask, and every
masked row OR column produces constant NEG output. So the device only ever
needs attn at (unmasked row) x (unmasked col):
  - host packs attn[b, h][ix_(rows=u, cols=u)] -> [NU, H_PER, C] (C = NU_max)
  - device computes attn + bias on the packed tile — NO mask op at all
    (every element of the computed region is unmasked)
  - device output row r (an unmasked row): [h0 C | h1 C | h2 C | h3 C |
    NEG tail 4*NMc_max] — the tail supplies the NEG bytes for that row's
    masked columns (4 heads' worth)
  - rows [NU, n_rows) are full-NEG rows for the masked rows
  - host unshard scatters computed bytes through the row/col permutations
    and the NEG bytes to the masked positions; every output byte originates
    on device.

All DMA transfers keep exactly 128 partitions (partial-partition DMAs
serialize onto one engine); the last block of each region is anchored at
(end-128) and the overlap is written twice with identical bytes.

Queue layout (per-packet round-robin arbitration weights by packet size):
  sync   — loads            (~17 KB packets)
  gpsimd — computed stores  (~34 KB packets, issued after DVE finishes)
  scalar — NEG row stores   (~34 KB packets, independent, interleaved)

Bitwise exactness: bias = round(BETA * round(cs_i*cs_j)) then band added in
reference order; adds are IEEE f32, identical to the jax reference.
"""

import sys

sys.path.insert(0, "/opt/trn_rl_repo")

from contextlib import ExitStack

import numpy as np

import concourse.bass as bass
import concourse.tile as tile
from concourse import bacc, mybir
from concourse.bass_utils import run_bass_kernel_spmd

ALPHA = np.float32(0.5)
BETA = np.float32(0.1)
NEG = np.float32(-100000.0)
BIG = np.float32(3.0e38)

B, H, L = 2, 16, 2048
N_CORES = 8
H_PER = (B * H) // N_CORES
P = 128
K = 5  # per-row values: cs, t1, v1, t2, v2

FP = mybir.dt.float32


def _block_starts(n, base=0):
    if n <= 0:
        return []
    if n <= P:
        return [base]
    starts = list(range(base, base + n - P, P))
    starts.append(base + n - P)
    return sorted(set(starts))


def _build_program(C, tail, n_rows, starts, m_starts, windows, trace_sim=False):
    nb = len(starts)
    W = H_PER * C + tail  # device out row width (f32 elems)
    wmax = max((wn for _, wn in windows), default=1)

    nc = bacc.Bacc(
        "TRN2",
        target_bir_lowering=False,
        debug=False,
        num_devices=N_CORES,
    )

    nu_buf = max((starts[-1] + P) if starts else P, P)
    attn_d = nc.dram_tensor("attn", [nu_buf, H_PER, C], FP, kind="ExternalInput").ap()
    vecs_d = nc.dram_tensor("vecs", [P, max(nb, 1) * K], FP, kind="ExternalInput").ap()
    # rowconsts[0] = packed-col c_sink, [1] = iota over packed cols
    rowconsts_d = nc.dram_tensor("rowconsts", [2, C], FP, kind="ExternalInput").ap()
    out_d = nc.dram_tensor("out", [n_rows, W], FP, kind="ExternalOutput").ap()

    attn_rr = attn_d.rearrange("r h c -> r (h c)")

    with tile.TileContext(nc, trace_sim=trace_sim) as tc, ExitStack() as ctx:
        const_pool = ctx.enter_context(tc.tile_pool(name="const", bufs=1))
        prep_pool = ctx.enter_context(tc.tile_pool(name="prep", bufs=2))
        band_pool = ctx.enter_context(tc.tile_pool(name="band", bufs=1))
        a_pool = ctx.enter_context(tc.tile_pool(name="a", bufs=3))

        vecs_sb = const_pool.tile([P, max(nb, 1) * K], FP, tag="vecs")
        nc.sync.dma_start(out=vecs_sb[:, :], in_=vecs_d[:, :])
        bcs = []
        for i in range(2):
            crow = band_pool.tile([1, C], FP, tag="crow")
            nc.sync.dma_start(out=crow[:, :], in_=rowconsts_d[i : i + 1, :])
            bc = const_pool.tile([P, C], FP, tag=f"bc{i}")
            nc.gpsimd.partition_broadcast(bc[:, :], crow[0:1, :])
            bcs.append(bc)
        csink_bc, iota_bc = bcs

        # constant NEG tile: full-NEG rows for the masked-row region
        neg_stores = []
        if m_starts:
            neg_t = const_pool.tile([P, W], FP, tag="neg")
            nc.gpsimd.memset(neg_t[:, :], float(NEG))
            neg_stores = [(out_d[r0 : r0 + P, :], neg_t[:, :]) for r0 in m_starts]
        # NEG stores interleave 1:1 with computed stores on the gpsimd
        # queue (in-order ring: all-upfront would block computed stores
        # behind 38 MB of NEG; scalar-queue NEG got scheduled late in v4)
        neg_per_block = [[] for _ in range(max(nb, 1))]
        for i, ns in enumerate(neg_stores):
            neg_per_block[(i * max(nb, 1)) // max(len(neg_stores), 1)].append(ns)
        if nb == 0:
            for dst, src in neg_stores:
                nc.gpsimd.dma_start(out=dst, in_=src)

        for r, k0 in enumerate(starts):
            cs_col = vecs_sb[:, K * r + 0 : K * r + 1]
            t1_col = vecs_sb[:, K * r + 1 : K * r + 2]
            v1_col = vecs_sb[:, K * r + 2 : K * r + 3]
            t2_col = vecs_sb[:, K * r + 3 : K * r + 4]
            v2_col = vecs_sb[:, K * r + 4 : K * r + 5]
            ws, wn = windows[r]

            bias_t = prep_pool.tile([P, C], FP, tag="bias")
            nc.scalar.activation(
                out=bias_t[:, :],
                in_=csink_bc[:, :],
                func=mybir.ActivationFunctionType.Copy,
                scale=cs_col,
            )
            nc.scalar.activation(
                out=bias_t[:, :],
                in_=bias_t[:, :],
                func=mybir.ActivationFunctionType.Copy,
                scale=float(BETA),
            )
            if wn > 0:
                band1 = band_pool.tile([P, wmax], FP, tag="band1")
                nc.vector.tensor_scalar(
                    out=band1[:, :wn],
                    in0=iota_bc[:, ws : ws + wn],
                    scalar1=t1_col,
                    scalar2=v1_col,
                    op0=mybir.AluOpType.is_equal,
                    op1=mybir.AluOpType.mult,
                )
                band2 = band_pool.tile([P, wmax], FP, tag="band2")
                nc.vector.tensor_scalar(
                    out=band2[:, :wn],
                    in0=iota_bc[:, ws : ws + wn],
                    scalar1=t2_col,
                    scalar2=v2_col,
                    op0=mybir.AluOpType.is_equal,
                    op1=mybir.AluOpType.mult,
                )
                bias_win = bias_t[:, ws : ws + wn]
                nc.vector.tensor_tensor(
                    out=bias_win, in0=bias_win, in1=band1[:, :wn],
                    op=mybir.AluOpType.add,
                )
                nc.vector.tensor_tensor(
                    out=bias_win, in0=bias_win, in1=band2[:, :wn],
                    op=mybir.AluOpType.add,
                )

            # a_t row: [4 heads x C computed | NEG tail]
            a_t = a_pool.tile([P, W], FP, tag="a")
            nc.sync.dma_start(
                out=a_t[:, : H_PER * C], in_=attn_rr[k0 : k0 + P, :]
            )
            if tail:
                nc.vector.memset(a_t[:, H_PER * C :], float(NEG))
            for h in range(H_PER):
                a_h = a_t[:, h * C : (h + 1) * C]
                nc.vector.tensor_tensor(
                    out=a_h, in0=a_h, in1=bias_t[:, :], op=mybir.AluOpType.add
                )
            nc.gpsimd.dma_start(out=out_d[k0 : k0 + P, :], in_=a_t[:, :])
            for dst, src in neg_per_block[r]:
                nc.gpsimd.dma_start(out=dst, in_=src)

    nc.compile()
    return nc


def _band_vecs(c_local_b):
    sub = np.zeros(L, np.float32)
    sub[1] = c_local_b[1]
    sub[L - 1] = c_local_b[L - 1]
    sub[2 : L - 1] = c_local_b[1 : L - 2]
    sup = np.zeros(L, np.float32)
    sup[: L - 1] = c_local_b[1:]
    return ALPHA * sub, ALPHA * sup


def _host_prep(attn_scores, c_local, c_sink, mask):
    attn_scores = np.asarray(attn_scores, dtype=np.float32)
    c_local = np.asarray(c_local, dtype=np.float32)
    c_sink = np.asarray(c_sink, dtype=np.float32)
    mask = np.asarray(mask, dtype=bool)

    unm = [np.flatnonzero(~mask[b]) for b in range(B)]
    msk = [np.flatnonzero(mask[b]) for b in range(B)]
    nu = [len(u) for u in unm]
    nu_max = max(nu)
    nm_max = max(L - n for n in nu)  # max masked count (rows == cols)
    C = max(nu_max, 1)
    tail = H_PER * nm_max  # per-row NEG bytes for 4 heads' masked cols
    nu_buf = max(nu_max, P)
    starts = _block_starts(nu_max)
    neg_base = nu_max if nu_max > P else (P if nu_max else 0)
    m_starts = _block_starts(max(nm_max, P if nm_max else 0), base=neg_base)
    n_rows = (m_starts[-1] + P) if m_starts else (starts[-1] + P if starts else 0)
    nb = len(starts)

    # padded row/col permutations (junk rows/cols; host ignores them)
    pis = []
    for b in range(B):
        pi = np.empty(nu_buf, np.int64)
        pi[: nu[b]] = unm[b]
        if nu[b] < nu_buf:
            pi[nu[b] :] = unm[b][-1] if nu[b] else 0
        pis.append(pi)
    pcs = []
    for b in range(B):
        pc = np.empty(C, np.int64)
        pc[: nu[b]] = unm[b][:C][: nu[b]]
        if nu[b] < C:
            pc[nu[b] :] = unm[b][-1] if nu[b] else 0
        pcs.append(pc)

    # band targets in packed-col space: q with u[q] == i +- 1, else -5
    # (a masked neighbor column never appears: its output is NEG anyway)
    tgt = []
    for b in range(B):
        lut = np.full(L + 2, -5, np.float32)
        lut[unm[b]] = np.arange(nu[b], dtype=np.float32)
        tgt.append(lut)

    windows = []
    for k0 in starts:
        ws, we = C, -1
        for b in range(B):
            rows = pis[b][k0 : k0 + P]
            for t in (tgt[b][np.maximum(rows - 1, 0)], tgt[b][np.minimum(rows + 1, L - 1)]):
                valid = t[t >= 0]
                if len(valid):
                    ws = min(ws, int(valid.min()))
                    we = max(we, int(valid.max()))
        windows.append((ws, we - ws + 1) if we >= ws else (0, 0))

    in_maps = []
    for c in range(N_CORES):
        b = c // (N_CORES // B)
        h0 = H_PER * (c % (N_CORES // B))
        pi, pc = pis[b], pcs[b]
        sub, sup = _band_vecs(c_local[b])

        vecs = np.zeros((P, max(nb, 1) * K), np.float32)
        for r, k0 in enumerate(starts):
            rows = pi[k0 : k0 + P]
            vecs[:, K * r + 0] = c_sink[b][rows]
            vecs[:, K * r + 1] = tgt[b][np.maximum(rows - 1, 0)]
            vecs[:, K * r + 1][rows == 0] = -5.0
            vecs[:, K * r + 2] = sub[rows]
            vecs[:, K * r + 3] = tgt[b][np.minimum(rows + 1, L - 1)]
            vecs[:, K * r + 3][rows == L - 1] = -5.0
            vecs[:, K * r + 4] = sup[rows]

        rowconsts = np.stack(
            [c_sink[b][pc], np.arange(C, dtype=np.float32)], axis=0
        )
        attn_packed = np.ascontiguousarray(
            attn_scores[b, h0 : h0 + H_PER][:, pi, :][:, :, pc].transpose(1, 0, 2)
        )
        in_maps.append(
            {
                "attn": attn_packed,
                "vecs": np.ascontiguousarray(vecs),
                "rowconsts": np.ascontiguousarray(rowconsts),
            }
        )
    shape_key = (
        C, tail, n_rows, tuple(starts), tuple(m_starts), tuple(windows)
    )
    return in_maps, shape_key, (unm, msk, nu, neg_base)


_PROGRAM_CACHE = {}


def kernel(attn_scores, c_local, c_sink, mask, _trace=False, _trace_kwargs=None):
    in_maps, shape_key, (unm, msk, nu, neg_base) = _host_prep(
        attn_scores, c_local, c_sink, mask
    )
    C, tail, n_rows, starts, m_starts, windows = shape_key
    if shape_key not in _PROGRAM_CACHE:
        _PROGRAM_CACHE.clear()
        _PROGRAM_CACHE[shape_key] = _build_program(
            C, tail, n_rows, list(starts), list(m_starts), list(windows)
        )
    nc = _PROGRAM_CACHE[shape_key]

    res = run_bass_kernel_spmd(
        nc,
        in_maps,
        list(range(N_CORES)),
        trace=_trace,
        **(_trace_kwargs or {}),
    )
    out = np.empty((B, H, L, L), dtype=np.float32)
    for c in range(N_CORES):
        b = c // (N_CORES // B)
        h0 = H_PER * (c % (N_CORES // B))
        dev = res.results[c]["out"]  # [n_rows, W]
        nub, nmb = nu[b], L - nu[b]
        u, m = unm[b], msk[b]
        for hh in range(H_PER):
            h = h0 + hh
            # computed block: rows u x cols u
            out[b, h][np.ix_(u, u)] = dev[:nub, hh * C : hh * C + nub]
            # masked cols of unmasked rows: NEG bytes from the row tails
            out[b, h][np.ix_(u, m)] = dev[
                :nub, H_PER * C + hh * (tail // H_PER) :
            ][:, :nmb]
            # masked rows: full-NEG device rows (W >= H_PER*L, so each
            # head takes a distinct byte range)
            out[b, h][m, :] = dev[neg_base : neg_base + nmb, hh * L : (hh + 1) * L]
    kernel.last_results = res
    return out
